# revision 5
# baseline (speedup 1.0000x reference)
"""Trainium2 Bass kernel for nn_DGG_LearnableK_Small.

The reference collapses analytically (see baseline notes):
  - softmax over a size-1 axis == 1, so log_p == 0 and edge_prob == 1/N
    exactly; stable argsort of a constant row is the identity permutation.
    idxs is therefore the input-independent constant iota [B,N,N] and is
    assembled on the host.
  - adj_hard[b,i,j] = sigmoid(cke - 7j + sum_l s_l relu(z_l + b1f_l)),
    z = x @ W1f, where the linear tail is folded on the host:
      wv7 = W2 @ (7 Wkp),  s = sign(wv7),  aw = |wv7|,
      W1f = W1*aw, b1f = b1*aw, cke = 2 + 7*(b2@Wkp + bkp).
    sigmoid underflows to exactly 0.0f for j >= CUT=16 at any plausible
    shift; only 16 adj columns are computed, the rest are host zeros.

Device program (per core, 1024 rows), transposed L-on-partition layout:
  PE:   4 z-matmuls  z[l, r] (lhsT = W1f chunk [128d,128l], rhs = xT
        [128d,512r], PSUM [128,512] f32) + 4 k-sum matmuls
        (lhsT = S16 [128l,16] = sign replicated 16x, rhs = y bf16) that
        both reduce over l AND broadcast the per-row logit shift to the
        16 output partitions: pk[i, r] = sum_l s_l y[l, r] for all i.
  DVE:  y = max(z, -b1f) per tile ([128,512] PSUM->SBUF bf16); the
        missing +b1f rotates into the sigmoid bias as
        C = sum_l s_l b1f_l (host constant).
  ACT:  2 sigmoids [16,512]: adjT = sigmoid(pk + bias), bias[j] =
        cke + C - 7j per-partition.  A dependency-free dummy sigmoid at
        the top of the ACT queue hoists the ACT_TABLE_LOADs off the
        critical path (they run during the input DMAs).
  DMA:  row-half 0 is computed first end-to-end (both its z matmuls only
        need the first px half), so sigmoid 0's bf16 output half leaves
        on the scalar ring while row-half 1 is still in flight.
"""

import os

import numpy as np

B, N, D, L = 4, 2048, 128, 256
NCORES = 8
ROWS = B * N          # 8192
RPC = ROWS // NCORES  # 1024 rows per core
P = 128
HALF = RPC // 2       # 512 rows per row-half (one PSUM bank of f32)
INTERVAL = 7.0
HS_START = 2.0
CUT = 16              # adj columns actually computed (rest stay 0)
LC = L // P           # 2 L-chunks of 128
PWC = L + LC * CUT    # pw tensor free size: W1f [128,256] + S16 [128,2*16]

VARIANT = os.environ.get("DGG_VARIANT", "dve4")

_CACHE = {}

# Results of the last device run (exec time etc.) for the local test harness.
LAST_RESULTS = None


def _build_nc(variant):
    import concourse.bacc as bacc
    import concourse.mybir as mybir
    from concourse.tile import TileContext

    f32 = mybir.dt.float32
    bf16 = mybir.dt.bfloat16
    AF = mybir.ActivationFunctionType

    # Bacc (not plain Bass): its compile() legalizes semaphore waits for the
    # TRN2 one-wait-per-instruction constraint via event semaphores.
    nc = bacc.Bacc(None, target_bir_lowering=False, debug=False)
    px = nc.declare_dram_parameter("px", [P, RPC], bf16, isOutput=False)
    pw = nc.declare_dram_parameter("pw", [P, PWC], bf16, isOutput=False)
    paux = nc.declare_dram_parameter("paux", [P, 8], f32, isOutput=False)
    adjT = nc.declare_dram_parameter("adjT", [CUT, RPC], bf16, isOutput=True)

    with TileContext(nc) as tc:
        with (
            tc.tile_pool(name="sb", bufs=1) as sbp,
            tc.tile_pool(name="ps", bufs=1, space="PSUM") as ppool,
        ):
            # Dependency-free dummy activation at the top of the ACT queue:
            # Bacc.insert_act_table_loads places the sigmoid table loads
            # right before it, so they overlap the input DMAs instead of
            # stalling the first real sigmoid.
            dsrc = sbp.tile([1, 1], f32, tag="dsrc")
            ddst = sbp.tile([1, 1], f32, tag="ddst")
            nc.vector.memset(dsrc, 0.0)
            nc.scalar.activation(ddst, dsrc, AF.Sigmoid)

            px_sb = sbp.tile([P, RPC], bf16, tag="px")
            pw_sb = sbp.tile([P, PWC], bf16, tag="pw")
            aux_sb = sbp.tile([P, 8], f32, tag="paux")
            # Input split over the three rings.  The Pool ring is software
            # DGE (slow start, last to deliver), so it carries the px
            # quarter the last z-matmul consumes.  aux rides first on SP.
            nc.sync.dma_start(out=aux_sb, in_=paux[:])
            nc.sync.dma_start(out=px_sb[:, 0:256], in_=px[:, 0:256])
            nc.scalar.dma_start(out=pw_sb, in_=pw[:])
            nc.gpsimd.dma_start(out=px_sb[:, 768:1024], in_=px[:, 768:1024])
            nc.sync.dma_start(out=px_sb[:, 256:512], in_=px[:, 256:512])
            nc.scalar.dma_start(out=px_sb[:, 512:768], in_=px[:, 512:768])

            # PSUM: four z banks + two pk banks.
            zt = [[ppool.tile([P, HALF], f32, name=f"z{c}{h}", tag=f"z{c}{h}")
                   for h in (0, 1)] for c in range(LC)]
            pk = [ppool.tile([P, HALF], f32, name=f"pk{h}", tag=f"pk{h}")
                  for h in (0, 1)]
            yt = [[sbp.tile([P, HALF], bf16, name=f"y{c}{h}", tag=f"y{c}{h}")
                   for h in (0, 1)] for c in range(LC)]
            out_sb = sbp.tile([CUT, RPC], bf16, tag="adjT")

            # Row-half 0 first, end to end: its two z-matmuls only need
            # the first px half, its sigmoid's output half can leave
            # while row-half 1 computes.
            for h in (0, 1):
                for c in range(LC):
                    nc.tensor.matmul(
                        zt[c][h],
                        lhsT=pw_sb[:, c * P:(c + 1) * P],
                        rhs=px_sb[:, h * HALF:(h + 1) * HALF],
                        start=True,
                        stop=True,
                        skip_group_check=True,
                    )
                for c in range(LC):
                    nc.vector.tensor_scalar_max(
                        yt[c][h], zt[c][h], aux_sb[:, 2 + c:3 + c])
                for c in range(LC):
                    nc.tensor.matmul(
                        pk[h][0:CUT, :],
                        lhsT=pw_sb[:, L + c * CUT:L + (c + 1) * CUT],
                        rhs=yt[c][h],
                        start=(c == 0),
                        stop=(c == LC - 1),
                        skip_group_check=True,
                    )
                nc.scalar.activation(
                    out_sb[:, h * HALF:(h + 1) * HALF],
                    pk[h][0:CUT, :],
                    AF.Sigmoid,
                    bias=aux_sb[0:CUT, 4 + h:5 + h],
                    scale=1.0,
                )
                # Each output half leaves on the ACT-sequencer ring in
                # queue order right after its sigmoid.
                nc.scalar.dma_start(
                    out=adjT[:, h * HALF:(h + 1) * HALF],
                    in_=out_sb[:, h * HALF:(h + 1) * HALF],
                )

    nc.compile()
    return nc


def kernel(**inputs):
    global LAST_RESULTS
    import ml_dtypes
    from concourse.bass_utils import run_bass_kernel_spmd

    bf16 = ml_dtypes.bfloat16

    x = np.ascontiguousarray(np.asarray(inputs["x"], dtype=np.float32))
    W1 = np.asarray(inputs["W_mu1"], dtype=np.float32)
    b1v = np.asarray(inputs["b_mu1"], dtype=np.float32)
    W2 = np.asarray(inputs["W_mu2"], dtype=np.float32)
    b2v = np.asarray(inputs["b_mu2"], dtype=np.float32)
    Wkp = np.asarray(inputs["W_kp"], dtype=np.float32)
    bkp = np.asarray(inputs["b_kp"], dtype=np.float32)

    # Host-side folding of the linear tail (replicated across cores).
    wv7 = (W2.astype(np.float64) @ (INTERVAL * Wkp[:, 0].astype(np.float64)))
    cke = HS_START + INTERVAL * float(
        b2v.astype(np.float64) @ Wkp[:, 0].astype(np.float64)
        + np.float64(bkp[0]))
    s = np.where(wv7 > 0, 1.0, -1.0)
    aw = np.abs(wv7)
    W1f = (W1.astype(np.float64) * aw[None, :]).astype(np.float32)
    b1f = (b1v.astype(np.float64) * aw).astype(np.float64)

    variant = VARIANT
    key = ("nc", variant)
    if key not in _CACHE:
        _CACHE[key] = _build_nc(variant)
    nc = _CACHE[key]

    # Sigmoid-bias correction: the max-form tiles drop +b1f, so
    # C = sum_l s_l b1f_l comes back via the per-partition bias.
    C = float((s * b1f).sum())

    pw_h = np.empty((P, PWC), dtype=bf16)
    pw_h[:, 0:L] = W1f.astype(bf16)
    for c in range(LC):
        pw_h[:, L + c * CUT:L + (c + 1) * CUT] = (
            s[c * P:(c + 1) * P].astype(bf16)[:, None])

    paux_h = np.zeros((P, 8), dtype=np.float32)
    paux_h[:, 0] = b1f[0:P].astype(np.float32)
    paux_h[:, 1] = b1f[P:2 * P].astype(np.float32)
    paux_h[:, 2] = (-b1f[0:P]).astype(np.float32)
    paux_h[:, 3] = (-b1f[P:2 * P]).astype(np.float32)
    js = np.arange(CUT, dtype=np.float64)
    paux_h[0:CUT, 4] = (cke + C - INTERVAL * js).astype(np.float32)
    paux_h[0:CUT, 5] = (cke + C - INTERVAL * js).astype(np.float32)

    x_flat = x.reshape(ROWS, D)
    in_maps = []
    for c in range(NCORES):
        pxc = np.ascontiguousarray(
            x_flat[c * RPC:(c + 1) * RPC].T).astype(bf16)
        in_maps.append({"px": pxc, "pw": pw_h, "paux": paux_h})

    try:
        res = run_bass_kernel_spmd(nc, in_maps, list(range(NCORES)))
    except ModuleNotFoundError:
        # BASS_TRACE was set in an environment without the axon NTFF hook
        # module; retry with tracing forced off.
        os.environ["BASS_NEVER_TRACE"] = "1"
        res = run_bass_kernel_spmd(nc, in_maps, list(range(NCORES)))
    LAST_RESULTS = res

    adj_full = np.zeros((ROWS, N), dtype=np.float32)
    for c in range(NCORES):
        adj_full[c * RPC:(c + 1) * RPC, 0:CUT] = (
            res.results[c]["adjT"].astype(np.float32).T)
    idx_full = np.broadcast_to(
        np.arange(N, dtype=np.int32), (B, N, N)).copy()

    return adj_full.reshape(B, N, N), idx_full


# revision 7
# speedup vs baseline: 1.0216x; 1.0216x over previous
"""Trainium2 Bass kernel for nn_DGG_LearnableK_Small.

The reference collapses analytically (see baseline notes):
  - softmax over a size-1 axis == 1, so log_p == 0 and edge_prob == 1/N
    exactly; stable argsort of a constant row is the identity permutation.
    idxs is therefore the input-independent constant iota [B,N,N] and is
    assembled on the host.
  - adj_hard[b,i,j] = sigmoid(cke - 7j + sum_l s_l relu(z_l + b1f_l)),
    z = x @ W1f, where the linear tail is folded on the host:
      wv7 = W2 @ (7 Wkp),  s = sign(wv7),  aw = |wv7|,
      W1f = W1*aw, b1f = b1*aw, cke = 2 + 7*(b2@Wkp + bkp).
    sigmoid underflows to exactly 0.0f for j >= CUT=16 at any plausible
    shift; only 16 adj columns are computed, the rest are host zeros.

Device program (per core, 1024 rows), transposed L-on-partition layout:
  PE:   4 z-matmuls  z[l, r] (lhsT = W1f chunk [128d,128l], rhs = xT
        [128d,512r], PSUM [128,512] f32) + 4 k-sum matmuls
        (lhsT = S16 [128l,16] = sign replicated 16x, rhs = y bf16) that
        both reduce over l AND broadcast the per-row logit shift to the
        16 output partitions: pk[i, r] = sum_l s_l y[l, r] for all i.
  DVE:  y = max(z, -b1f) per tile ([128,512] PSUM->SBUF bf16); the
        missing +b1f rotates into the sigmoid bias as
        C = sum_l s_l b1f_l (host constant).
  ACT:  2 sigmoids [16,512]: adjT = sigmoid(pk + bias), bias[j] =
        cke + C - 7j per-partition.  A dependency-free dummy sigmoid at
        the top of the ACT queue hoists the ACT_TABLE_LOADs off the
        critical path (they run during the input DMAs).
  DMA:  row-half 0 is computed first end-to-end (both its z matmuls only
        need the first px half), so sigmoid 0's bf16 output half leaves
        on the scalar ring while row-half 1 is still in flight.
"""

import os

import numpy as np

B, N, D, L = 4, 2048, 128, 256
NCORES = 8
ROWS = B * N          # 8192
RPC = ROWS // NCORES  # 1024 rows per core
P = 128
HALF = RPC // 2       # 512 rows per row-half (one PSUM bank of f32)
INTERVAL = 7.0
HS_START = 2.0
CUT = 16              # adj columns actually computed (rest stay 0)
LC = L // P           # 2 L-chunks of 128
PWC = L + LC * CUT    # pw tensor free size: W1f [128,256] + S16 [128,2*16]

VARIANT = os.environ.get("DGG_VARIANT", "dve4")

_CACHE = {}

# Results of the last device run (exec time etc.) for the local test harness.
LAST_RESULTS = None


def _build_nc(variant):
    import concourse.bacc as bacc
    import concourse.mybir as mybir
    from concourse.tile import TileContext

    f32 = mybir.dt.float32
    bf16 = mybir.dt.bfloat16
    AF = mybir.ActivationFunctionType

    # Bacc (not plain Bass): its compile() legalizes semaphore waits for the
    # TRN2 one-wait-per-instruction constraint via event semaphores.
    nc = bacc.Bacc(None, target_bir_lowering=False, debug=False)
    px = nc.declare_dram_parameter("px", [P, RPC], bf16, isOutput=False)
    pw = nc.declare_dram_parameter("pw", [P, PWC], bf16, isOutput=False)
    paux = nc.declare_dram_parameter("paux", [P, 8], f32, isOutput=False)
    adjT = nc.declare_dram_parameter("adjT", [CUT, RPC], bf16, isOutput=True)

    with TileContext(nc) as tc:
        with (
            tc.tile_pool(name="sb", bufs=1) as sbp,
            tc.tile_pool(name="ps", bufs=1, space="PSUM") as ppool,
        ):
            # Dependency-free dummy activation at the top of the ACT queue:
            # Bacc.insert_act_table_loads places the sigmoid table loads
            # right before it, so they overlap the input DMAs instead of
            # stalling the first real sigmoid.
            dsrc = sbp.tile([1, 1], f32, tag="dsrc")
            ddst = sbp.tile([1, 1], f32, tag="ddst")
            nc.vector.memset(dsrc, 0.0)
            nc.scalar.activation(ddst, dsrc, AF.Sigmoid)

            px_sb = sbp.tile([P, RPC], bf16, tag="px")
            pw_sb = sbp.tile([P, PWC], bf16, tag="pw")
            aux_sb = sbp.tile([P, 8], f32, tag="paux")
            # DMA queue feed costs ~45ns/descriptor (16 per dma_start)
            # regardless of size, so inputs go as three big transfers:
            # the px half that gates both row-0 z-matmuls leads on SP,
            # weights + the second px half on the scalar ring, and the
            # small aux vector rides the (slow, software-DGE) Pool ring.
            nc.sync.dma_start(out=px_sb[:, 0:HALF], in_=px[:, 0:HALF])
            nc.scalar.dma_start(out=pw_sb, in_=pw[:])
            nc.gpsimd.dma_start(out=aux_sb, in_=paux[:])
            nc.scalar.dma_start(out=px_sb[:, HALF:RPC], in_=px[:, HALF:RPC])

            # PSUM: four z banks + two pk banks.
            zt = [[ppool.tile([P, HALF], f32, name=f"z{c}{h}", tag=f"z{c}{h}")
                   for h in (0, 1)] for c in range(LC)]
            pk = [ppool.tile([P, HALF], f32, name=f"pk{h}", tag=f"pk{h}")
                  for h in (0, 1)]
            yt = [[sbp.tile([P, HALF], bf16, name=f"y{c}{h}", tag=f"y{c}{h}")
                   for h in (0, 1)] for c in range(LC)]
            out_sb = sbp.tile([CUT, RPC], bf16, tag="adjT")

            # Row-half 0 first, end to end: its two z-matmuls only need
            # the first px half, its sigmoid's output half can leave
            # while row-half 1 computes.
            for h in (0, 1):
                for c in range(LC):
                    nc.tensor.matmul(
                        zt[c][h],
                        lhsT=pw_sb[:, c * P:(c + 1) * P],
                        rhs=px_sb[:, h * HALF:(h + 1) * HALF],
                        start=True,
                        stop=True,
                        skip_group_check=True,
                    )
                for c in range(LC):
                    nc.vector.tensor_scalar_max(
                        yt[c][h], zt[c][h], aux_sb[:, 2 + c:3 + c])
                for c in range(LC):
                    nc.tensor.matmul(
                        pk[h][0:CUT, :],
                        lhsT=pw_sb[:, L + c * CUT:L + (c + 1) * CUT],
                        rhs=yt[c][h],
                        start=(c == 0),
                        stop=(c == LC - 1),
                        skip_group_check=True,
                    )
                nc.scalar.activation(
                    out_sb[:, h * HALF:(h + 1) * HALF],
                    pk[h][0:CUT, :],
                    AF.Sigmoid,
                    bias=aux_sb[0:CUT, 4 + h:5 + h],
                    scale=1.0,
                )
                # Output halves leave on the idle SP ring: a dma feed on
                # the ACT queue between the sigmoids would stall sigmoid 1.
                nc.sync.dma_start(
                    out=adjT[:, h * HALF:(h + 1) * HALF],
                    in_=out_sb[:, h * HALF:(h + 1) * HALF],
                )

    nc.compile()
    return nc


def kernel(**inputs):
    global LAST_RESULTS
    import ml_dtypes
    from concourse.bass_utils import run_bass_kernel_spmd

    bf16 = ml_dtypes.bfloat16

    x = np.ascontiguousarray(np.asarray(inputs["x"], dtype=np.float32))
    W1 = np.asarray(inputs["W_mu1"], dtype=np.float32)
    b1v = np.asarray(inputs["b_mu1"], dtype=np.float32)
    W2 = np.asarray(inputs["W_mu2"], dtype=np.float32)
    b2v = np.asarray(inputs["b_mu2"], dtype=np.float32)
    Wkp = np.asarray(inputs["W_kp"], dtype=np.float32)
    bkp = np.asarray(inputs["b_kp"], dtype=np.float32)

    # Host-side folding of the linear tail (replicated across cores).
    wv7 = (W2.astype(np.float64) @ (INTERVAL * Wkp[:, 0].astype(np.float64)))
    cke = HS_START + INTERVAL * float(
        b2v.astype(np.float64) @ Wkp[:, 0].astype(np.float64)
        + np.float64(bkp[0]))
    s = np.where(wv7 > 0, 1.0, -1.0)
    aw = np.abs(wv7)
    W1f = (W1.astype(np.float64) * aw[None, :]).astype(np.float32)
    b1f = (b1v.astype(np.float64) * aw).astype(np.float64)

    variant = VARIANT
    key = ("nc", variant)
    if key not in _CACHE:
        _CACHE[key] = _build_nc(variant)
    nc = _CACHE[key]

    # Sigmoid-bias correction: the max-form tiles drop +b1f, so
    # C = sum_l s_l b1f_l comes back via the per-partition bias.
    C = float((s * b1f).sum())

    pw_h = np.empty((P, PWC), dtype=bf16)
    pw_h[:, 0:L] = W1f.astype(bf16)
    for c in range(LC):
        pw_h[:, L + c * CUT:L + (c + 1) * CUT] = (
            s[c * P:(c + 1) * P].astype(bf16)[:, None])

    paux_h = np.zeros((P, 8), dtype=np.float32)
    paux_h[:, 0] = b1f[0:P].astype(np.float32)
    paux_h[:, 1] = b1f[P:2 * P].astype(np.float32)
    paux_h[:, 2] = (-b1f[0:P]).astype(np.float32)
    paux_h[:, 3] = (-b1f[P:2 * P]).astype(np.float32)
    js = np.arange(CUT, dtype=np.float64)
    paux_h[0:CUT, 4] = (cke + C - INTERVAL * js).astype(np.float32)
    paux_h[0:CUT, 5] = (cke + C - INTERVAL * js).astype(np.float32)

    x_flat = x.reshape(ROWS, D)
    in_maps = []
    for c in range(NCORES):
        pxc = np.ascontiguousarray(
            x_flat[c * RPC:(c + 1) * RPC].T).astype(bf16)
        in_maps.append({"px": pxc, "pw": pw_h, "paux": paux_h})

    try:
        res = run_bass_kernel_spmd(nc, in_maps, list(range(NCORES)))
    except ModuleNotFoundError:
        # BASS_TRACE was set in an environment without the axon NTFF hook
        # module; retry with tracing forced off.
        os.environ["BASS_NEVER_TRACE"] = "1"
        res = run_bass_kernel_spmd(nc, in_maps, list(range(NCORES)))
    LAST_RESULTS = res

    adj_full = np.zeros((ROWS, N), dtype=np.float32)
    for c in range(NCORES):
        adj_full[c * RPC:(c + 1) * RPC, 0:CUT] = (
            res.results[c]["adjT"].astype(np.float32).T)
    idx_full = np.broadcast_to(
        np.arange(N, dtype=np.int32), (B, N, N)).copy()

    return adj_full.reshape(B, N, N), idx_full


# revision 11
# speedup vs baseline: 1.0564x; 1.0340x over previous
"""Trainium2 Bass kernel for nn_DGG_LearnableK_Small.

The reference collapses analytically (see baseline notes):
  - softmax over a size-1 axis == 1, so log_p == 0 and edge_prob == 1/N
    exactly; stable argsort of a constant row is the identity permutation.
    idxs is therefore the input-independent constant iota [B,N,N] and is
    assembled on the host.
  - adj_hard[b,i,j] = sigmoid(cke - 7j + sum_l s_l relu(z_l + b1f_l)),
    z = x @ W1f, where the linear tail is folded on the host:
      wv7 = W2 @ (7 Wkp),  s = sign(wv7),  aw = |wv7|,
      W1f = W1*aw, b1f = b1*aw, cke = 2 + 7*(b2@Wkp + bkp).
    sigmoid underflows to exactly 0.0f for j >= CUT=16 at any plausible
    shift; only 16 adj columns are computed, the rest are host zeros.

Device program (per core, 1024 rows), transposed L-on-partition layout:
  PE:   4 z-matmuls  z[l, r] (lhsT = W1f chunk [128d,128l], rhs = xT
        [128d,512r], PSUM [128,512] f32) + 4 k-sum matmuls
        (lhsT = S16 [128l,16] = sign replicated 16x, rhs = y bf16) that
        both reduce over l AND broadcast the per-row logit shift to the
        16 output partitions: pk[i, r] = sum_l s_l y[l, r] for all i.
  DVE:  y = max(z, -b1f) per tile ([128,512] PSUM->SBUF bf16); the
        missing +b1f rotates into the sigmoid bias as
        C = sum_l s_l b1f_l (host constant).
  ACT:  2 sigmoids [16,512]: adjT = sigmoid(pk + bias), bias[j] =
        cke + C - 7j per-partition.  A dependency-free dummy sigmoid at
        the top of the ACT queue hoists the ACT_TABLE_LOADs off the
        critical path (they run during the input DMAs).
  DMA:  row-half 0 is computed first end-to-end (both its z matmuls only
        need the first px half), so sigmoid 0's bf16 output half leaves
        on the scalar ring while row-half 1 is still in flight.
"""

import os

import numpy as np

B, N, D, L = 4, 2048, 128, 256
NCORES = 8
ROWS = B * N          # 8192
RPC = ROWS // NCORES  # 1024 rows per core
P = 128
HALF = RPC // 2       # 512 rows per row-half (one PSUM bank of f32)
INTERVAL = 7.0
HS_START = 2.0
CUT = 16              # adj columns actually computed (rest stay 0)
LC = L // P           # 2 L-chunks of 128
PWC = L + LC * CUT    # pw tensor free size: W1f [128,256] + S16 [128,2*16]

VARIANT = os.environ.get("DGG_VARIANT", "dve4")

# (chunk, rowhalf) y tiles computed on ACT as exact relu(z+b) instead of
# DVE max(z,-b); chosen to shorten the serial DVE chain feeding sigmoid 1.
ACT_RELU = {(0, 1)}

_CACHE = {}

# Results of the last device run (exec time etc.) for the local test harness.
LAST_RESULTS = None


def _build_nc(variant):
    import concourse.bacc as bacc
    import concourse.mybir as mybir
    from concourse.tile import TileContext

    f32 = mybir.dt.float32
    bf16 = mybir.dt.bfloat16
    AF = mybir.ActivationFunctionType

    # Bacc (not plain Bass): its compile() legalizes semaphore waits for the
    # TRN2 one-wait-per-instruction constraint via event semaphores.
    nc = bacc.Bacc(None, target_bir_lowering=False, debug=False)
    px = nc.declare_dram_parameter("px", [P, RPC], bf16, isOutput=False)
    pw = nc.declare_dram_parameter("pw", [P, PWC], bf16, isOutput=False)
    paux = nc.declare_dram_parameter("paux", [P, 8], f32, isOutput=False)
    adjT = nc.declare_dram_parameter("adjT", [CUT, RPC], bf16, isOutput=True)

    with TileContext(nc) as tc:
        with (
            tc.tile_pool(name="sb", bufs=1) as sbp,
            tc.tile_pool(name="ps", bufs=1, space="PSUM") as ppool,
        ):
            # Dependency-free dummy activation at the top of the ACT queue:
            # Bacc.insert_act_table_loads places the sigmoid table loads
            # right before it, so they overlap the input DMAs instead of
            # stalling the first real sigmoid.
            dsrc = sbp.tile([1, 1], f32, tag="dsrc")
            ddst = sbp.tile([1, 1], f32, tag="ddst")
            nc.vector.memset(dsrc, 0.0)
            nc.scalar.activation(ddst, dsrc, AF.Sigmoid)

            px_sb = sbp.tile([P, RPC], bf16, tag="px")
            pw_sb = sbp.tile([P, PWC], bf16, tag="pw")
            aux_sb = sbp.tile([P, 8], f32, tag="paux")
            # DMA queue feed costs ~45ns/descriptor (16 per dma_start)
            # regardless of size, so inputs go as three big transfers:
            # the px half that gates both row-0 z-matmuls leads on SP,
            # weights + the second px half on the scalar ring, and the
            # small aux vector rides the (slow, software-DGE) Pool ring.
            nc.sync.dma_start(out=px_sb[:, 0:HALF], in_=px[:, 0:HALF])
            nc.scalar.dma_start(out=pw_sb, in_=pw[:])
            nc.gpsimd.dma_start(out=aux_sb, in_=paux[:])
            nc.scalar.dma_start(out=px_sb[:, HALF:RPC], in_=px[:, HALF:RPC])

            # PSUM: four z banks + two pk banks.
            zt = [[ppool.tile([P, HALF], f32, name=f"z{c}{h}", tag=f"z{c}{h}")
                   for h in (0, 1)] for c in range(LC)]
            pk = [ppool.tile([P, HALF], f32, name=f"pk{h}", tag=f"pk{h}")
                  for h in (0, 1)]
            yt = [[sbp.tile([P, HALF], bf16, name=f"y{c}{h}", tag=f"y{c}{h}")
                   for h in (0, 1)] for c in range(LC)]
            out_sb = sbp.tile([CUT, RPC], bf16, tag="adjT")

            # All z-matmuls up front (row-half 0 first: it only needs the
            # first px half), then the y tiles, then the k-sums.  The y
            # chain is the serial tail, so one tile rides the otherwise
            # idle ACT engine as an exact relu(z+b) while DVE does the
            # max-form ones (ACT_RELU below marks which).
            for h in (0, 1):
                for c in range(LC):
                    nc.tensor.matmul(
                        zt[c][h],
                        lhsT=pw_sb[:, c * P:(c + 1) * P],
                        rhs=px_sb[:, h * HALF:(h + 1) * HALF],
                        start=True,
                        stop=True,
                        skip_group_check=True,
                    )
            for h, c in ((0, 0), (0, 1), (1, 0), (1, 1)):
                if (c, h) in ACT_RELU:
                    nc.scalar.activation(
                        yt[c][h], zt[c][h], AF.Relu,
                        bias=aux_sb[:, c:c + 1], scale=1.0)
                else:
                    nc.vector.tensor_scalar_max(
                        yt[c][h], zt[c][h], aux_sb[:, 2 + c:3 + c])
            for h in (0, 1):
                for c in range(LC):
                    nc.tensor.matmul(
                        pk[h][0:CUT, :],
                        lhsT=pw_sb[:, L + c * CUT:L + (c + 1) * CUT],
                        rhs=yt[c][h],
                        start=(c == 0),
                        stop=(c == LC - 1),
                        skip_group_check=True,
                    )
                nc.scalar.activation(
                    out_sb[:, h * HALF:(h + 1) * HALF],
                    pk[h][0:CUT, :],
                    AF.Sigmoid,
                    bias=aux_sb[0:CUT, 4 + h:5 + h],
                    scale=1.0,
                )
                # Output halves leave on the idle SP ring: a dma feed on
                # the ACT queue between the sigmoids would stall sigmoid 1.
                nc.sync.dma_start(
                    out=adjT[:, h * HALF:(h + 1) * HALF],
                    in_=out_sb[:, h * HALF:(h + 1) * HALF],
                )

    nc.compile()
    return nc


def kernel(**inputs):
    global LAST_RESULTS
    import ml_dtypes
    from concourse.bass_utils import run_bass_kernel_spmd

    bf16 = ml_dtypes.bfloat16

    x = np.ascontiguousarray(np.asarray(inputs["x"], dtype=np.float32))
    W1 = np.asarray(inputs["W_mu1"], dtype=np.float32)
    b1v = np.asarray(inputs["b_mu1"], dtype=np.float32)
    W2 = np.asarray(inputs["W_mu2"], dtype=np.float32)
    b2v = np.asarray(inputs["b_mu2"], dtype=np.float32)
    Wkp = np.asarray(inputs["W_kp"], dtype=np.float32)
    bkp = np.asarray(inputs["b_kp"], dtype=np.float32)

    # Host-side folding of the linear tail (replicated across cores).
    wv7 = (W2.astype(np.float64) @ (INTERVAL * Wkp[:, 0].astype(np.float64)))
    cke = HS_START + INTERVAL * float(
        b2v.astype(np.float64) @ Wkp[:, 0].astype(np.float64)
        + np.float64(bkp[0]))
    s = np.where(wv7 > 0, 1.0, -1.0)
    aw = np.abs(wv7)
    W1f = (W1.astype(np.float64) * aw[None, :]).astype(np.float32)
    b1f = (b1v.astype(np.float64) * aw).astype(np.float64)

    variant = VARIANT
    key = ("nc", variant)
    if key not in _CACHE:
        _CACHE[key] = _build_nc(variant)
    nc = _CACHE[key]

    # Sigmoid-bias correction: the max-form tiles drop +b1f, so the
    # per-row-half C_h = sum over max-form chunks of s*b1f comes back
    # via the per-partition bias.
    csb = [float((s * b1f)[c * P:(c + 1) * P].sum()) for c in range(LC)]
    Ch = [sum(csb[c] for c in range(LC) if (c, h) not in ACT_RELU)
          for h in (0, 1)]

    pw_h = np.empty((P, PWC), dtype=bf16)
    pw_h[:, 0:L] = W1f.astype(bf16)
    for c in range(LC):
        pw_h[:, L + c * CUT:L + (c + 1) * CUT] = (
            s[c * P:(c + 1) * P].astype(bf16)[:, None])

    paux_h = np.zeros((P, 8), dtype=np.float32)
    paux_h[:, 0] = b1f[0:P].astype(np.float32)
    paux_h[:, 1] = b1f[P:2 * P].astype(np.float32)
    paux_h[:, 2] = (-b1f[0:P]).astype(np.float32)
    paux_h[:, 3] = (-b1f[P:2 * P]).astype(np.float32)
    js = np.arange(CUT, dtype=np.float64)
    paux_h[0:CUT, 4] = (cke + Ch[0] - INTERVAL * js).astype(np.float32)
    paux_h[0:CUT, 5] = (cke + Ch[1] - INTERVAL * js).astype(np.float32)

    x_flat = x.reshape(ROWS, D)
    in_maps = []
    for c in range(NCORES):
        pxc = np.ascontiguousarray(
            x_flat[c * RPC:(c + 1) * RPC].T).astype(bf16)
        in_maps.append({"px": pxc, "pw": pw_h, "paux": paux_h})

    try:
        res = run_bass_kernel_spmd(nc, in_maps, list(range(NCORES)))
    except ModuleNotFoundError:
        # BASS_TRACE was set in an environment without the axon NTFF hook
        # module; retry with tracing forced off.
        os.environ["BASS_NEVER_TRACE"] = "1"
        res = run_bass_kernel_spmd(nc, in_maps, list(range(NCORES)))
    LAST_RESULTS = res

    adj_full = np.zeros((ROWS, N), dtype=np.float32)
    for c in range(NCORES):
        adj_full[c * RPC:(c + 1) * RPC, 0:CUT] = (
            res.results[c]["adjT"].astype(np.float32).T)
    idx_full = np.broadcast_to(
        np.arange(N, dtype=np.int32), (B, N, N)).copy()

    return adj_full.reshape(B, N, N), idx_full


# revision 15
# speedup vs baseline: 1.1695x; 1.1071x over previous
"""Trainium2 Bass kernel for nn_DGG_LearnableK_Small.

The reference collapses analytically (see baseline notes):
  - softmax over a size-1 axis == 1, so log_p == 0 and edge_prob == 1/N
    exactly; stable argsort of a constant row is the identity permutation.
    idxs is therefore the input-independent constant iota [B,N,N] and is
    assembled on the host.
  - adj_hard[b,i,j] = sigmoid(cke - 7j + sum_l s_l relu(z_l + b1f_l)),
    z = x @ W1f, where the linear tail is folded on the host:
      wv7 = W2 @ (7 Wkp),  s = sign(wv7),  aw = |wv7|,
      W1f = W1*aw, b1f = b1*aw, cke = 2 + 7*(b2@Wkp + bkp).
    sigmoid underflows to exactly 0.0f for j >= CUT=16 at any plausible
    shift; only 16 adj columns are computed, the rest are host zeros.

Device program (per core, 1024 rows), transposed L-on-partition layout:
  PE:   4 z-matmuls  z[l, r] (lhsT = W1f chunk [128d,128l], rhs = xT
        [128d,512r], PSUM [128,512] f32) + 4 k-sum matmuls
        (lhsT = S16 [128l,16] = sign replicated 16x, rhs = y bf16) that
        both reduce over l AND broadcast the per-row logit shift to the
        16 output partitions: pk[i, r] = sum_l s_l y[l, r] for all i.
  DVE:  y = max(z, -b1f) per tile ([128,512] PSUM->SBUF bf16); the
        missing +b1f rotates into the sigmoid bias as
        C = sum_l s_l b1f_l (host constant).
  ACT:  2 sigmoids [16,512]: adjT = sigmoid(pk + bias), bias[j] =
        cke + C - 7j per-partition.  A dependency-free dummy sigmoid at
        the top of the ACT queue hoists the ACT_TABLE_LOADs off the
        critical path (they run during the input DMAs).
  DMA:  row-half 0 is computed first end-to-end (both its z matmuls only
        need the first px half), so sigmoid 0's bf16 output half leaves
        on the scalar ring while row-half 1 is still in flight.
"""

import os

import numpy as np

B, N, D, L = 4, 2048, 128, 256
NCORES = 8
ROWS = B * N          # 8192
RPC = ROWS // NCORES  # 1024 rows per core
P = 128
HALF = RPC // 2       # 512 rows per row-half (one PSUM bank of f32)
INTERVAL = 7.0
HS_START = 2.0
CUT = 16              # adj columns actually computed (rest stay 0)
LC = L // P           # 2 L-chunks of 128
PWC = L + LC * CUT    # pw tensor free size: W1f [128,256] + S16 [128,2*16]

VARIANT = os.environ.get("DGG_VARIANT", "raw")

# (chunk, rowhalf) y tiles computed on ACT as exact relu(z+b) instead of
# DVE max(z,-b); chosen to shorten the serial DVE chain feeding sigmoid 1.
ACT_RELU = {(0, 1)}

_CACHE = {}

# Results of the last device run (exec time etc.) for the local test harness.
LAST_RESULTS = None


def _build_raw():
    """Hand-scheduled raw-Bass build: no TileContext, so no pool entry/exit
    barriers, and the input DMA feeds issue at window start.  Every
    cross-engine hazard is covered by one dedicated semaphore and every
    instruction carries at most one wait (no event-semaphore legalization).
    """
    import concourse.bacc as bacc
    import concourse.mybir as mybir

    f32 = mybir.dt.float32
    bf16 = mybir.dt.bfloat16
    AF = mybir.ActivationFunctionType

    nc = bacc.Bacc(None, target_bir_lowering=False, debug=False)
    px = nc.declare_dram_parameter("px", [P, RPC], bf16, isOutput=False)
    pw = nc.declare_dram_parameter("pw", [P, PWC], bf16, isOutput=False)
    paux = nc.declare_dram_parameter("paux", [P, 8], f32, isOutput=False)
    adjT = nc.declare_dram_parameter("adjT", [CUT, RPC], bf16, isOutput=True)

    px_sb = nc.alloc_sbuf_tensor("px_sb", [P, RPC], bf16)
    pw_sb = nc.alloc_sbuf_tensor("pw_sb", [P, PWC], bf16)
    aux_sb = nc.alloc_sbuf_tensor("aux_sb", [P, 8], f32)
    yt = [[nc.alloc_sbuf_tensor(f"y{c}{h}", [P, HALF], bf16) for h in (0, 1)]
          for c in range(LC)]
    out_sb = nc.alloc_sbuf_tensor("out_sb", [CUT, RPC], bf16)
    dsc = nc.alloc_sbuf_tensor("dsc", [1, 2], f32)

    zt = [[nc.alloc_psum_tensor(f"z{c}{h}", [P, HALF], f32) for h in (0, 1)]
          for c in range(LC)]
    pk = [nc.alloc_psum_tensor(f"pk{h}", [P, HALF], f32) for h in (0, 1)]

    s_pxA = nc.alloc_semaphore("s_pxA")
    s_pxB = nc.alloc_semaphore("s_pxB")
    s_pw = nc.alloc_semaphore("s_pw")
    s_aux = nc.alloc_semaphore("s_aux")
    s_z = nc.alloc_semaphore("s_z")
    s_yd = nc.alloc_semaphore("s_yd")
    s_ya = nc.alloc_semaphore("s_ya")
    s_pk = nc.alloc_semaphore("s_pk")
    s_sig = nc.alloc_semaphore("s_sig")
    s_out = nc.alloc_semaphore("s_out")

    # ACT queue.  Dependency-free dummy activations first: the table-load
    # pass puts both ACT_TABLE_LOADs before them, overlapping the DMAs.
    # (dsc is read uninitialized on purpose; the result is scratch.)
    nc.scalar.activation(dsc[0:1, 1:2], dsc[0:1, 0:1], AF.Sigmoid)
    nc.scalar.activation(dsc[0:1, 1:2], dsc[0:1, 0:1], AF.Relu)
    nc.scalar.dma_start(out=pw_sb[:], in_=pw[:]).then_inc(s_pw, 16)
    nc.scalar.dma_start(
        out=px_sb[:, HALF:RPC], in_=px[:, HALF:RPC]).then_inc(s_pxB, 16)

    # SP queue: the px half that gates both row-0 z-matmuls, then later
    # the two output halves (fed as each sigmoid finishes).
    nc.sync.dma_start(
        out=px_sb[:, 0:HALF], in_=px[:, 0:HALF]).then_inc(s_pxA, 16)

    # Pool (software-DGE) queue: the small aux vector.
    nc.gpsimd.dma_start(out=aux_sb[:], in_=paux[:]).then_inc(s_aux, 16)

    # PE queue.
    nc.tensor.wait_ge(s_pw, 16)
    nc.tensor.wait_ge(s_pxA, 16)
    order = ((0, 0), (1, 0), (0, 1), (1, 1))
    for c, h in order:
        if (c, h) == (0, 1):
            nc.tensor.wait_ge(s_pxB, 16)
        nc.tensor.matmul(
            zt[c][h][:],
            lhsT=pw_sb[:, c * P:(c + 1) * P],
            rhs=px_sb[:, h * HALF:(h + 1) * HALF],
            start=True,
            stop=True,
            skip_group_check=True,
        ).then_inc(s_z, 1)
    # k-sums: pk0 (rows 0) first so sigmoid 0 can start early.
    nc.tensor.wait_ge(s_yd, 1)
    nc.tensor.matmul(pk[0][0:CUT, :], lhsT=pw_sb[:, L:L + CUT],
                     rhs=yt[0][0][:], start=True, stop=False,
                     skip_group_check=True)
    nc.tensor.wait_ge(s_yd, 2)
    nc.tensor.matmul(pk[0][0:CUT, :], lhsT=pw_sb[:, L + CUT:L + 2 * CUT],
                     rhs=yt[1][0][:], start=False, stop=True,
                     skip_group_check=True).then_inc(s_pk, 1)
    nc.tensor.wait_ge(s_ya, 1)
    nc.tensor.matmul(pk[1][0:CUT, :], lhsT=pw_sb[:, L:L + CUT],
                     rhs=yt[0][1][:], start=True, stop=False,
                     skip_group_check=True)
    nc.tensor.wait_ge(s_yd, 3)
    nc.tensor.matmul(pk[1][0:CUT, :], lhsT=pw_sb[:, L + CUT:L + 2 * CUT],
                     rhs=yt[1][1][:], start=False, stop=True,
                     skip_group_check=True).then_inc(s_pk, 1)

    # DVE queue: the three max-form y tiles in pk order (z-sem counts
    # follow the PE z order above: z00=1, z10=2, z01=3, z11=4).
    nc.vector.wait_ge(s_aux, 16)
    nc.vector.wait_ge(s_z, 1)
    nc.vector.tensor_scalar_max(
        yt[0][0][:], zt[0][0][:], aux_sb[:, 2:3]).then_inc(s_yd, 1)
    nc.vector.wait_ge(s_z, 2)
    nc.vector.tensor_scalar_max(
        yt[1][0][:], zt[1][0][:], aux_sb[:, 3:4]).then_inc(s_yd, 1)
    nc.vector.wait_ge(s_z, 4)
    nc.vector.tensor_scalar_max(
        yt[1][1][:], zt[1][1][:], aux_sb[:, 3:4]).then_inc(s_yd, 1)

    # ACT queue (continued): exact relu for tile (0,1), then the sigmoids.
    nc.scalar.wait_ge(s_aux, 16)
    nc.scalar.wait_ge(s_z, 3)
    nc.scalar.activation(
        yt[0][1][:], zt[0][1][:], AF.Relu,
        bias=aux_sb[:, 0:1], scale=1.0).then_inc(s_ya, 1)
    for h in (0, 1):
        nc.scalar.wait_ge(s_pk, h + 1)
        nc.scalar.activation(
            out_sb[:, h * HALF:(h + 1) * HALF],
            pk[h][0:CUT, :],
            AF.Sigmoid,
            bias=aux_sb[0:CUT, 4 + h:5 + h],
            scale=1.0,
        ).then_inc(s_sig, 1)

    # SP queue (continued): output halves leave as soon as each sigmoid
    # lands; the final wait holds the program open until the data is out.
    for h in (0, 1):
        nc.sync.wait_ge(s_sig, h + 1)
        nc.sync.dma_start(
            out=adjT[:, h * HALF:(h + 1) * HALF],
            in_=out_sb[:, h * HALF:(h + 1) * HALF],
        ).then_inc(s_out, 16)
    nc.sync.wait_ge(s_out, 32)

    nc.compile()
    return nc


def _build_nc(variant):
    import concourse.bacc as bacc
    import concourse.mybir as mybir
    from concourse.tile import TileContext

    f32 = mybir.dt.float32
    bf16 = mybir.dt.bfloat16
    AF = mybir.ActivationFunctionType

    # Bacc (not plain Bass): its compile() legalizes semaphore waits for the
    # TRN2 one-wait-per-instruction constraint via event semaphores.
    nc = bacc.Bacc(None, target_bir_lowering=False, debug=False)
    px = nc.declare_dram_parameter("px", [P, RPC], bf16, isOutput=False)
    pw = nc.declare_dram_parameter("pw", [P, PWC], bf16, isOutput=False)
    paux = nc.declare_dram_parameter("paux", [P, 8], f32, isOutput=False)
    adjT = nc.declare_dram_parameter("adjT", [CUT, RPC], bf16, isOutput=True)

    with TileContext(nc) as tc:
        with (
            tc.tile_pool(name="sb", bufs=1) as sbp,
            tc.tile_pool(name="ps", bufs=1, space="PSUM") as ppool,
        ):
            # Dependency-free dummy activation at the top of the ACT queue:
            # Bacc.insert_act_table_loads places the sigmoid table loads
            # right before it, so they overlap the input DMAs instead of
            # stalling the first real sigmoid.
            dsrc = sbp.tile([1, 1], f32, tag="dsrc")
            ddst = sbp.tile([1, 1], f32, tag="ddst")
            nc.vector.memset(dsrc, 0.0)
            nc.scalar.activation(ddst, dsrc, AF.Sigmoid)

            px_sb = sbp.tile([P, RPC], bf16, tag="px")
            pw_sb = sbp.tile([P, PWC], bf16, tag="pw")
            aux_sb = sbp.tile([P, 8], f32, tag="paux")
            # DMA queue feed costs ~45ns/descriptor (16 per dma_start)
            # regardless of size, so inputs go as three big transfers:
            # the px half that gates both row-0 z-matmuls leads on SP,
            # weights + the second px half on the scalar ring, and the
            # small aux vector rides the (slow, software-DGE) Pool ring.
            nc.sync.dma_start(out=px_sb[:, 0:HALF], in_=px[:, 0:HALF])
            nc.scalar.dma_start(out=pw_sb, in_=pw[:])
            nc.gpsimd.dma_start(out=aux_sb, in_=paux[:])
            nc.scalar.dma_start(out=px_sb[:, HALF:RPC], in_=px[:, HALF:RPC])

            # PSUM: four z banks + two pk banks.
            zt = [[ppool.tile([P, HALF], f32, name=f"z{c}{h}", tag=f"z{c}{h}")
                   for h in (0, 1)] for c in range(LC)]
            pk = [ppool.tile([P, HALF], f32, name=f"pk{h}", tag=f"pk{h}")
                  for h in (0, 1)]
            yt = [[sbp.tile([P, HALF], bf16, name=f"y{c}{h}", tag=f"y{c}{h}")
                   for h in (0, 1)] for c in range(LC)]
            out_sb = sbp.tile([CUT, RPC], bf16, tag="adjT")

            # All z-matmuls up front (row-half 0 first: it only needs the
            # first px half), then the y tiles, then the k-sums.  The y
            # chain is the serial tail, so one tile rides the otherwise
            # idle ACT engine as an exact relu(z+b) while DVE does the
            # max-form ones (ACT_RELU below marks which).
            for h in (0, 1):
                for c in range(LC):
                    nc.tensor.matmul(
                        zt[c][h],
                        lhsT=pw_sb[:, c * P:(c + 1) * P],
                        rhs=px_sb[:, h * HALF:(h + 1) * HALF],
                        start=True,
                        stop=True,
                        skip_group_check=True,
                    )
            for h, c in ((0, 0), (0, 1), (1, 0), (1, 1)):
                if (c, h) in ACT_RELU:
                    nc.scalar.activation(
                        yt[c][h], zt[c][h], AF.Relu,
                        bias=aux_sb[:, c:c + 1], scale=1.0)
                else:
                    nc.vector.tensor_scalar_max(
                        yt[c][h], zt[c][h], aux_sb[:, 2 + c:3 + c])
            for h in (0, 1):
                for c in range(LC):
                    nc.tensor.matmul(
                        pk[h][0:CUT, :],
                        lhsT=pw_sb[:, L + c * CUT:L + (c + 1) * CUT],
                        rhs=yt[c][h],
                        start=(c == 0),
                        stop=(c == LC - 1),
                        skip_group_check=True,
                    )
                nc.scalar.activation(
                    out_sb[:, h * HALF:(h + 1) * HALF],
                    pk[h][0:CUT, :],
                    AF.Sigmoid,
                    bias=aux_sb[0:CUT, 4 + h:5 + h],
                    scale=1.0,
                )
                # Output halves leave on the idle SP ring: a dma feed on
                # the ACT queue between the sigmoids would stall sigmoid 1.
                nc.sync.dma_start(
                    out=adjT[:, h * HALF:(h + 1) * HALF],
                    in_=out_sb[:, h * HALF:(h + 1) * HALF],
                )

    nc.compile()
    return nc


def kernel(**inputs):
    global LAST_RESULTS
    import ml_dtypes
    from concourse.bass_utils import run_bass_kernel_spmd

    bf16 = ml_dtypes.bfloat16

    x = np.ascontiguousarray(np.asarray(inputs["x"], dtype=np.float32))
    W1 = np.asarray(inputs["W_mu1"], dtype=np.float32)
    b1v = np.asarray(inputs["b_mu1"], dtype=np.float32)
    W2 = np.asarray(inputs["W_mu2"], dtype=np.float32)
    b2v = np.asarray(inputs["b_mu2"], dtype=np.float32)
    Wkp = np.asarray(inputs["W_kp"], dtype=np.float32)
    bkp = np.asarray(inputs["b_kp"], dtype=np.float32)

    # Host-side folding of the linear tail (replicated across cores).
    wv7 = (W2.astype(np.float64) @ (INTERVAL * Wkp[:, 0].astype(np.float64)))
    cke = HS_START + INTERVAL * float(
        b2v.astype(np.float64) @ Wkp[:, 0].astype(np.float64)
        + np.float64(bkp[0]))
    s = np.where(wv7 > 0, 1.0, -1.0)
    aw = np.abs(wv7)
    W1f = (W1.astype(np.float64) * aw[None, :]).astype(np.float32)
    b1f = (b1v.astype(np.float64) * aw).astype(np.float64)

    variant = VARIANT
    key = ("nc", variant)
    if key not in _CACHE:
        _CACHE[key] = (_build_raw() if variant == "raw"
                       else _build_nc(variant))
    nc = _CACHE[key]

    # Sigmoid-bias correction: the max-form tiles drop +b1f, so the
    # per-row-half C_h = sum over max-form chunks of s*b1f comes back
    # via the per-partition bias.
    csb = [float((s * b1f)[c * P:(c + 1) * P].sum()) for c in range(LC)]
    Ch = [sum(csb[c] for c in range(LC) if (c, h) not in ACT_RELU)
          for h in (0, 1)]

    pw_h = np.empty((P, PWC), dtype=bf16)
    pw_h[:, 0:L] = W1f.astype(bf16)
    for c in range(LC):
        pw_h[:, L + c * CUT:L + (c + 1) * CUT] = (
            s[c * P:(c + 1) * P].astype(bf16)[:, None])

    paux_h = np.zeros((P, 8), dtype=np.float32)
    paux_h[:, 0] = b1f[0:P].astype(np.float32)
    paux_h[:, 1] = b1f[P:2 * P].astype(np.float32)
    paux_h[:, 2] = (-b1f[0:P]).astype(np.float32)
    paux_h[:, 3] = (-b1f[P:2 * P]).astype(np.float32)
    js = np.arange(CUT, dtype=np.float64)
    paux_h[0:CUT, 4] = (cke + Ch[0] - INTERVAL * js).astype(np.float32)
    paux_h[0:CUT, 5] = (cke + Ch[1] - INTERVAL * js).astype(np.float32)

    x_flat = x.reshape(ROWS, D)
    in_maps = []
    for c in range(NCORES):
        pxc = np.ascontiguousarray(
            x_flat[c * RPC:(c + 1) * RPC].T).astype(bf16)
        in_maps.append({"px": pxc, "pw": pw_h, "paux": paux_h})

    try:
        res = run_bass_kernel_spmd(nc, in_maps, list(range(NCORES)))
    except ModuleNotFoundError:
        # BASS_TRACE was set in an environment without the axon NTFF hook
        # module; retry with tracing forced off.
        os.environ["BASS_NEVER_TRACE"] = "1"
        res = run_bass_kernel_spmd(nc, in_maps, list(range(NCORES)))
    LAST_RESULTS = res

    adj_full = np.zeros((ROWS, N), dtype=np.float32)
    for c in range(NCORES):
        adj_full[c * RPC:(c + 1) * RPC, 0:CUT] = (
            res.results[c]["adjT"].astype(np.float32).T)
    idx_full = np.broadcast_to(
        np.arange(N, dtype=np.int32), (B, N, N)).copy()

    return adj_full.reshape(B, N, N), idx_full


# revision 17
# speedup vs baseline: 1.2789x; 1.0935x over previous
"""Trainium2 Bass kernel for nn_DGG_LearnableK_Small.

The reference collapses analytically (see baseline notes):
  - softmax over a size-1 axis == 1, so log_p == 0 and edge_prob == 1/N
    exactly; stable argsort of a constant row is the identity permutation.
    idxs is therefore the input-independent constant iota [B,N,N] and is
    assembled on the host.
  - adj_hard[b,i,j] = sigmoid(cke - 7j + sum_l s_l relu(z_l + b1f_l)),
    z = x @ W1f, where the linear tail is folded on the host:
      wv7 = W2 @ (7 Wkp),  s = sign(wv7),  aw = |wv7|,
      W1f = W1*aw, b1f = b1*aw, cke = 2 + 7*(b2@Wkp + bkp).
    sigmoid underflows to exactly 0.0f for j >= CUT=16 at any plausible
    shift; only 16 adj columns are computed, the rest are host zeros.

Device program (per core, 1024 rows), transposed L-on-partition layout:
  PE:   4 z-matmuls  z[l, r] (lhsT = W1f chunk [128d,128l], rhs = xT
        [128d,512r], PSUM [128,512] f32) + 4 k-sum matmuls
        (lhsT = S16 [128l,16] = sign replicated 16x, rhs = y bf16) that
        both reduce over l AND broadcast the per-row logit shift to the
        16 output partitions: pk[i, r] = sum_l s_l y[l, r] for all i.
  DVE:  y = max(z, -b1f) per tile ([128,512] PSUM->SBUF bf16); the
        missing +b1f rotates into the sigmoid bias as
        C = sum_l s_l b1f_l (host constant).
  ACT:  2 sigmoids [16,512]: adjT = sigmoid(pk + bias), bias[j] =
        cke + C - 7j per-partition.  A dependency-free dummy sigmoid at
        the top of the ACT queue hoists the ACT_TABLE_LOADs off the
        critical path (they run during the input DMAs).
  DMA:  row-half 0 is computed first end-to-end (both its z matmuls only
        need the first px half), so sigmoid 0's bf16 output half leaves
        on the scalar ring while row-half 1 is still in flight.
"""

import os

import numpy as np

B, N, D, L = 4, 2048, 128, 256
NCORES = 8
ROWS = B * N          # 8192
RPC = ROWS // NCORES  # 1024 rows per core
P = 128
HALF = RPC // 2       # 512 rows per row-half (one PSUM bank of f32)
INTERVAL = 7.0
HS_START = 2.0
CUT = 16              # adj columns actually computed (rest stay 0)
LC = L // P           # 2 L-chunks of 128
PWC = L + LC * CUT    # pw tensor free size: W1f [128,256] + S16 [128,2*16]

VARIANT = os.environ.get("DGG_VARIANT", "raw")

# (chunk, rowhalf) y tiles computed on ACT as exact relu(z+b) instead of
# DVE max(z,-b); chosen to shorten the serial DVE chain feeding sigmoid 1.
ACT_RELU = {(0, 1)}

_CACHE = {}

# Results of the last device run (exec time etc.) for the local test harness.
LAST_RESULTS = None


def _build_raw():
    """Hand-scheduled raw-Bass build: no TileContext, so no pool entry/exit
    barriers, and the input DMA feeds issue at window start.  Every
    cross-engine hazard is covered by one dedicated semaphore and every
    instruction carries at most one wait (no event-semaphore legalization).
    """
    import concourse.bacc as bacc
    import concourse.mybir as mybir

    f32 = mybir.dt.float32
    bf16 = mybir.dt.bfloat16
    AF = mybir.ActivationFunctionType

    nc = bacc.Bacc(None, target_bir_lowering=False, debug=False)
    px = nc.declare_dram_parameter("px", [P, RPC], bf16, isOutput=False)
    pw = nc.declare_dram_parameter("pw", [P, PWC], bf16, isOutput=False)
    paux = nc.declare_dram_parameter("paux", [P, 8], f32, isOutput=False)
    adjT = nc.declare_dram_parameter("adjT", [CUT, RPC], bf16, isOutput=True)

    px_sb = nc.alloc_sbuf_tensor("px_sb", [P, RPC], bf16)
    pw_sb = nc.alloc_sbuf_tensor("pw_sb", [P, PWC], bf16)
    aux_sb = nc.alloc_sbuf_tensor("aux_sb", [P, 8], f32)
    yt = [[nc.alloc_sbuf_tensor(f"y{c}{h}", [P, HALF], bf16) for h in (0, 1)]
          for c in range(LC)]
    out_sb = nc.alloc_sbuf_tensor("out_sb", [CUT, RPC], bf16)
    dsc = nc.alloc_sbuf_tensor("dsc", [1, 2], f32)

    zt = [[nc.alloc_psum_tensor(f"z{c}{h}", [P, HALF], f32) for h in (0, 1)]
          for c in range(LC)]
    pk = [nc.alloc_psum_tensor(f"pk{h}", [P, HALF], f32) for h in (0, 1)]

    s_pxA = nc.alloc_semaphore("s_pxA")
    s_pxB = nc.alloc_semaphore("s_pxB")
    s_pw = nc.alloc_semaphore("s_pw")
    s_aux = nc.alloc_semaphore("s_aux")
    s_z = nc.alloc_semaphore("s_z")
    s_yd = nc.alloc_semaphore("s_yd")
    s_ya = nc.alloc_semaphore("s_ya")
    s_pk = nc.alloc_semaphore("s_pk")
    s_sig = nc.alloc_semaphore("s_sig")
    s_out = nc.alloc_semaphore("s_out")

    # ACT queue.  Dependency-free dummy activations first: the table-load
    # pass puts both ACT_TABLE_LOADs before them, overlapping the DMAs.
    # (dsc is read uninitialized on purpose; the result is scratch.)
    nc.scalar.activation(dsc[0:1, 1:2], dsc[0:1, 0:1], AF.Sigmoid)
    nc.scalar.activation(dsc[0:1, 1:2], dsc[0:1, 0:1], AF.Relu)
    nc.scalar.dma_start(out=pw_sb[:], in_=pw[:]).then_inc(s_pw, 16)
    nc.scalar.dma_start(
        out=px_sb[:, HALF:RPC], in_=px[:, HALF:RPC]).then_inc(s_pxB, 16)

    # SP queue: the px half that gates both row-0 z-matmuls, then later
    # the two output halves (fed as each sigmoid finishes).
    nc.sync.dma_start(
        out=px_sb[:, 0:HALF], in_=px[:, 0:HALF]).then_inc(s_pxA, 16)

    # Pool (software-DGE) queue: the small aux vector.
    nc.gpsimd.dma_start(out=aux_sb[:], in_=paux[:]).then_inc(s_aux, 16)

    # PE queue.
    nc.tensor.wait_ge(s_pw, 16)
    nc.tensor.wait_ge(s_pxA, 16)
    order = ((0, 0), (1, 0), (0, 1), (1, 1))
    for c, h in order:
        if (c, h) == (0, 1):
            nc.tensor.wait_ge(s_pxB, 16)
        nc.tensor.matmul(
            zt[c][h][:],
            lhsT=pw_sb[:, c * P:(c + 1) * P],
            rhs=px_sb[:, h * HALF:(h + 1) * HALF],
            start=True,
            stop=True,
            skip_group_check=True,
        ).then_inc(s_z, 1)
    # k-sums: pk0 (rows 0) first so sigmoid 0 can start early; pk1 runs
    # its S1 term first so the S1 ldweights is shared with pk0's.
    nc.tensor.wait_ge(s_yd, 1)
    nc.tensor.matmul(pk[0][0:CUT, :], lhsT=pw_sb[:, L:L + CUT],
                     rhs=yt[0][0][:], start=True, stop=False,
                     skip_group_check=True)
    nc.tensor.wait_ge(s_yd, 2)
    nc.tensor.matmul(pk[0][0:CUT, :], lhsT=pw_sb[:, L + CUT:L + 2 * CUT],
                     rhs=yt[1][0][:], start=False, stop=True,
                     skip_group_check=True).then_inc(s_pk, 1)
    nc.tensor.wait_ge(s_yd, 3)
    nc.tensor.matmul(pk[1][0:CUT, :], lhsT=pw_sb[:, L + CUT:L + 2 * CUT],
                     rhs=yt[1][1][:], start=True, stop=False,
                     skip_group_check=True)
    nc.tensor.wait_ge(s_ya, 1)
    nc.tensor.matmul(pk[1][0:CUT, :], lhsT=pw_sb[:, L:L + CUT],
                     rhs=yt[0][1][:], start=False, stop=True,
                     skip_group_check=True).then_inc(s_pk, 1)

    # DVE queue: the three max-form y tiles in pk order (z-sem counts
    # follow the PE z order above: z00=1, z10=2, z01=3, z11=4).
    nc.vector.wait_ge(s_aux, 16)
    nc.vector.wait_ge(s_z, 1)
    nc.vector.tensor_scalar_max(
        yt[0][0][:], zt[0][0][:], aux_sb[:, 2:3]).then_inc(s_yd, 1)
    nc.vector.wait_ge(s_z, 2)
    nc.vector.tensor_scalar_max(
        yt[1][0][:], zt[1][0][:], aux_sb[:, 3:4]).then_inc(s_yd, 1)
    nc.vector.wait_ge(s_z, 4)
    nc.vector.tensor_scalar_max(
        yt[1][1][:], zt[1][1][:], aux_sb[:, 3:4]).then_inc(s_yd, 1)

    # ACT queue (continued): exact relu for tile (0,1), then the sigmoids.
    nc.scalar.wait_ge(s_aux, 16)
    nc.scalar.wait_ge(s_z, 3)
    nc.scalar.activation(
        yt[0][1][:], zt[0][1][:], AF.Relu,
        bias=aux_sb[:, 0:1], scale=1.0).then_inc(s_ya, 1)
    for h in (0, 1):
        nc.scalar.wait_ge(s_pk, h + 1)
        nc.scalar.activation(
            out_sb[:, h * HALF:(h + 1) * HALF],
            pk[h][0:CUT, :],
            AF.Sigmoid,
            bias=aux_sb[0:CUT, 4 + h:5 + h],
            scale=1.0,
        ).then_inc(s_sig, 1)

    # SP queue (continued): output halves leave as soon as each sigmoid
    # lands.  No trailing data-drain wait: the NRT postamble's
    # sync_barrier + dma_rearm quiesce the rings before execution is
    # reported complete (verified against the flush-waiting variant).
    for h in (0, 1):
        nc.sync.wait_ge(s_sig, h + 1)
        nc.sync.dma_start(
            out=adjT[:, h * HALF:(h + 1) * HALF],
            in_=out_sb[:, h * HALF:(h + 1) * HALF],
        ).then_inc(s_out, 16)

    # Drop the framework's kernel-entry all-engine barrier: it only
    # guards the const-AP memsets (which nothing here races with — all
    # activation biases are APs, the dummies' results are scratch) and
    # NRT's own preamble already zeroes the semaphores.  Removing it
    # lets the input DMA feeds issue at window start.
    for blk in nc.m.functions[0].blocks:
        blk.instructions = [
            i for i in blk.instructions
            if "barrier_Pool_Activation_PE_DVE_SP" not in i.concise()
        ]

    nc.compile()
    return nc


def _build_nc(variant):
    import concourse.bacc as bacc
    import concourse.mybir as mybir
    from concourse.tile import TileContext

    f32 = mybir.dt.float32
    bf16 = mybir.dt.bfloat16
    AF = mybir.ActivationFunctionType

    # Bacc (not plain Bass): its compile() legalizes semaphore waits for the
    # TRN2 one-wait-per-instruction constraint via event semaphores.
    nc = bacc.Bacc(None, target_bir_lowering=False, debug=False)
    px = nc.declare_dram_parameter("px", [P, RPC], bf16, isOutput=False)
    pw = nc.declare_dram_parameter("pw", [P, PWC], bf16, isOutput=False)
    paux = nc.declare_dram_parameter("paux", [P, 8], f32, isOutput=False)
    adjT = nc.declare_dram_parameter("adjT", [CUT, RPC], bf16, isOutput=True)

    with TileContext(nc) as tc:
        with (
            tc.tile_pool(name="sb", bufs=1) as sbp,
            tc.tile_pool(name="ps", bufs=1, space="PSUM") as ppool,
        ):
            # Dependency-free dummy activation at the top of the ACT queue:
            # Bacc.insert_act_table_loads places the sigmoid table loads
            # right before it, so they overlap the input DMAs instead of
            # stalling the first real sigmoid.
            dsrc = sbp.tile([1, 1], f32, tag="dsrc")
            ddst = sbp.tile([1, 1], f32, tag="ddst")
            nc.vector.memset(dsrc, 0.0)
            nc.scalar.activation(ddst, dsrc, AF.Sigmoid)

            px_sb = sbp.tile([P, RPC], bf16, tag="px")
            pw_sb = sbp.tile([P, PWC], bf16, tag="pw")
            aux_sb = sbp.tile([P, 8], f32, tag="paux")
            # DMA queue feed costs ~45ns/descriptor (16 per dma_start)
            # regardless of size, so inputs go as three big transfers:
            # the px half that gates both row-0 z-matmuls leads on SP,
            # weights + the second px half on the scalar ring, and the
            # small aux vector rides the (slow, software-DGE) Pool ring.
            nc.sync.dma_start(out=px_sb[:, 0:HALF], in_=px[:, 0:HALF])
            nc.scalar.dma_start(out=pw_sb, in_=pw[:])
            nc.gpsimd.dma_start(out=aux_sb, in_=paux[:])
            nc.scalar.dma_start(out=px_sb[:, HALF:RPC], in_=px[:, HALF:RPC])

            # PSUM: four z banks + two pk banks.
            zt = [[ppool.tile([P, HALF], f32, name=f"z{c}{h}", tag=f"z{c}{h}")
                   for h in (0, 1)] for c in range(LC)]
            pk = [ppool.tile([P, HALF], f32, name=f"pk{h}", tag=f"pk{h}")
                  for h in (0, 1)]
            yt = [[sbp.tile([P, HALF], bf16, name=f"y{c}{h}", tag=f"y{c}{h}")
                   for h in (0, 1)] for c in range(LC)]
            out_sb = sbp.tile([CUT, RPC], bf16, tag="adjT")

            # All z-matmuls up front (row-half 0 first: it only needs the
            # first px half), then the y tiles, then the k-sums.  The y
            # chain is the serial tail, so one tile rides the otherwise
            # idle ACT engine as an exact relu(z+b) while DVE does the
            # max-form ones (ACT_RELU below marks which).
            for h in (0, 1):
                for c in range(LC):
                    nc.tensor.matmul(
                        zt[c][h],
                        lhsT=pw_sb[:, c * P:(c + 1) * P],
                        rhs=px_sb[:, h * HALF:(h + 1) * HALF],
                        start=True,
                        stop=True,
                        skip_group_check=True,
                    )
            for h, c in ((0, 0), (0, 1), (1, 0), (1, 1)):
                if (c, h) in ACT_RELU:
                    nc.scalar.activation(
                        yt[c][h], zt[c][h], AF.Relu,
                        bias=aux_sb[:, c:c + 1], scale=1.0)
                else:
                    nc.vector.tensor_scalar_max(
                        yt[c][h], zt[c][h], aux_sb[:, 2 + c:3 + c])
            for h in (0, 1):
                for c in range(LC):
                    nc.tensor.matmul(
                        pk[h][0:CUT, :],
                        lhsT=pw_sb[:, L + c * CUT:L + (c + 1) * CUT],
                        rhs=yt[c][h],
                        start=(c == 0),
                        stop=(c == LC - 1),
                        skip_group_check=True,
                    )
                nc.scalar.activation(
                    out_sb[:, h * HALF:(h + 1) * HALF],
                    pk[h][0:CUT, :],
                    AF.Sigmoid,
                    bias=aux_sb[0:CUT, 4 + h:5 + h],
                    scale=1.0,
                )
                # Output halves leave on the idle SP ring: a dma feed on
                # the ACT queue between the sigmoids would stall sigmoid 1.
                nc.sync.dma_start(
                    out=adjT[:, h * HALF:(h + 1) * HALF],
                    in_=out_sb[:, h * HALF:(h + 1) * HALF],
                )

    nc.compile()
    return nc


def kernel(**inputs):
    global LAST_RESULTS
    import ml_dtypes
    from concourse.bass_utils import run_bass_kernel_spmd

    bf16 = ml_dtypes.bfloat16

    x = np.ascontiguousarray(np.asarray(inputs["x"], dtype=np.float32))
    W1 = np.asarray(inputs["W_mu1"], dtype=np.float32)
    b1v = np.asarray(inputs["b_mu1"], dtype=np.float32)
    W2 = np.asarray(inputs["W_mu2"], dtype=np.float32)
    b2v = np.asarray(inputs["b_mu2"], dtype=np.float32)
    Wkp = np.asarray(inputs["W_kp"], dtype=np.float32)
    bkp = np.asarray(inputs["b_kp"], dtype=np.float32)

    # Host-side folding of the linear tail (replicated across cores).
    wv7 = (W2.astype(np.float64) @ (INTERVAL * Wkp[:, 0].astype(np.float64)))
    cke = HS_START + INTERVAL * float(
        b2v.astype(np.float64) @ Wkp[:, 0].astype(np.float64)
        + np.float64(bkp[0]))
    s = np.where(wv7 > 0, 1.0, -1.0)
    aw = np.abs(wv7)
    W1f = (W1.astype(np.float64) * aw[None, :]).astype(np.float32)
    b1f = (b1v.astype(np.float64) * aw).astype(np.float64)

    variant = VARIANT
    key = ("nc", variant)
    if key not in _CACHE:
        _CACHE[key] = (_build_raw() if variant == "raw"
                       else _build_nc(variant))
    nc = _CACHE[key]

    # Sigmoid-bias correction: the max-form tiles drop +b1f, so the
    # per-row-half C_h = sum over max-form chunks of s*b1f comes back
    # via the per-partition bias.
    csb = [float((s * b1f)[c * P:(c + 1) * P].sum()) for c in range(LC)]
    Ch = [sum(csb[c] for c in range(LC) if (c, h) not in ACT_RELU)
          for h in (0, 1)]

    pw_h = np.empty((P, PWC), dtype=bf16)
    pw_h[:, 0:L] = W1f.astype(bf16)
    for c in range(LC):
        pw_h[:, L + c * CUT:L + (c + 1) * CUT] = (
            s[c * P:(c + 1) * P].astype(bf16)[:, None])

    paux_h = np.zeros((P, 8), dtype=np.float32)
    paux_h[:, 0] = b1f[0:P].astype(np.float32)
    paux_h[:, 1] = b1f[P:2 * P].astype(np.float32)
    paux_h[:, 2] = (-b1f[0:P]).astype(np.float32)
    paux_h[:, 3] = (-b1f[P:2 * P]).astype(np.float32)
    js = np.arange(CUT, dtype=np.float64)
    paux_h[0:CUT, 4] = (cke + Ch[0] - INTERVAL * js).astype(np.float32)
    paux_h[0:CUT, 5] = (cke + Ch[1] - INTERVAL * js).astype(np.float32)

    x_flat = x.reshape(ROWS, D)
    in_maps = []
    for c in range(NCORES):
        pxc = np.ascontiguousarray(
            x_flat[c * RPC:(c + 1) * RPC].T).astype(bf16)
        in_maps.append({"px": pxc, "pw": pw_h, "paux": paux_h})

    try:
        res = run_bass_kernel_spmd(nc, in_maps, list(range(NCORES)))
    except ModuleNotFoundError:
        # BASS_TRACE was set in an environment without the axon NTFF hook
        # module; retry with tracing forced off.
        os.environ["BASS_NEVER_TRACE"] = "1"
        res = run_bass_kernel_spmd(nc, in_maps, list(range(NCORES)))
    LAST_RESULTS = res

    adj_full = np.zeros((ROWS, N), dtype=np.float32)
    for c in range(NCORES):
        adj_full[c * RPC:(c + 1) * RPC, 0:CUT] = (
            res.results[c]["adjT"].astype(np.float32).T)
    idx_full = np.broadcast_to(
        np.arange(N, dtype=np.int32), (B, N, N)).copy()

    return adj_full.reshape(B, N, N), idx_full
